# revision 1
# baseline (speedup 1.0000x reference)
"""Trainium2 Bass kernel for AttentionDecoupleMetric (OAM).

Computation per batch b of x[b] in R^[C=512, P=784] (channels-major, the
native DRAM layout of the [B, C, H, W] input):

    D[p, q] = sum_c |x[c, p] - x[c, q]|          (symmetric, pairwise L1)
    s[p]    = sum_q D[p, q]
    Dn      = diag(1/s) @ D                      (row L1-normalized)
    M       = Dn^10 @ (ones(P)/P)                -> output [P]

Key algebraic reductions used here:
  * Dn^10 @ u is computed as 10 mat-vecs, not matrix powers.
  * With z_k := s * v_k, the iteration v' = diag(1/s) D v becomes
    z' = (D diag(1/s)) z with z_0 = s/P and M = z_10 / s, so the
    per-iteration normalization disappears (column-scaled D precomputed).
  * |a-b| = a + b - 2*min(a,b), so  D = S 1^T + 1 S^T - 2*MinSum  with
    S[p] = sum_c x[c,p] and MinSum[q,p] = sum_c min(x[c,p], x[c,q]).
    MinSum is symmetric -> only the strictly-upper triangle is computed
    (ONE bf16 DVE tensor_scalar `min` op per (q, channel-chunk), 4x mode),
    then mirrored via PE block transposes.  The rank-2 S corrections fold
    into existing elementwise passes for free.
  * sum_c runs on the TensorEngine: channels sit on partitions and a
    "ones at column j" weight matrix routes each q's reduction into PSUM
    row j while the min tile streams through as the moving operand
    (bf16 -> 1 col/cycle, fp32 PSUM accumulation).

bf16 effects on D wash out below 1e-6 in the 10x power smoothing (the
row-stochastic matrix mixes to near-uniform); everything downstream of
PSUM stays fp32.

Sharding: pure data-parallel, batch dim 16 -> 8 cores x 2 batches.
"""

import numpy as np

B, C, H, W = 16, 512, 28, 28
NP = H * W            # 784 positions
N_CORES = 8
BPC = B // N_CORES    # batches per core
P = 128               # partitions
NCC = C // P          # 4 channel chunks
NPB = 7               # position blocks (6 full + 1 of 16)
TAIL = NP - 6 * P     # 16
NP2 = NPB * P         # 896: D block-grid width (cols 784:896 stay zero)
N_ITER = 10
# q-columns with q % ACT_MOD >= ACT_CUT compute their min-tiles on the
# Activation engine (2 ops) instead of DVE (1 op), offloading the DVE
ACT_MOD, ACT_CUT = 8, 7

_CACHE = {}


def _build_program(repeat: int = 1):
    from contextlib import ExitStack

    import concourse.bacc as bacc
    import concourse.mybir as mybir
    import concourse.tile as tile
    from concourse.alu_op_type import AluOpType
    from concourse.masks import make_identity

    f32 = mybir.dt.float32
    bf16 = mybir.dt.bfloat16
    X = mybir.AxisListType.X
    Relu = mybir.ActivationFunctionType.Relu
    Ident = mybir.ActivationFunctionType.Identity

    nc = bacc.Bacc(
        "TRN2", target_bir_lowering=False, debug=False, num_devices=N_CORES
    )
    x_d = nc.dram_tensor("x", [BPC, C, NP], f32, kind="ExternalInput").ap()
    out_d = nc.dram_tensor("out", [BPC, NP], f32, kind="ExternalOutput").ap()

    def rcnt(i):  # valid row count of position block i
        return P if i < 6 else TAIL

    def blk(i):  # 128-wide column slice of position block i
        return slice(i * P, (i + 1) * P)

    with tile.TileContext(nc) as tc, ExitStack() as ctx:
        consts = ctx.enter_context(tc.tile_pool(name="consts", bufs=1))
        xpool = ctx.enter_context(tc.tile_pool(name="xpool", bufs=2))
        apool = ctx.enter_context(tc.tile_pool(name="apool", bufs=12))
        dpool = ctx.enter_context(tc.tile_pool(name="dpool", bufs=2))
        spool = ctx.enter_context(tc.tile_pool(name="spool", bufs=2))
        zpool = ctx.enter_context(tc.tile_pool(name="zpool", bufs=3))
        psum = ctx.enter_context(tc.tile_pool(name="psum", bufs=2, space="PSUM"))
        tpsum = ctx.enter_context(tc.tile_pool(name="tpsum", bufs=2, space="PSUM"))
        zpsum = ctx.enter_context(tc.tile_pool(name="zpsum", bufs=2, space="PSUM"))

        # Sliding-window weight buffer: W_j = Z[:, 128-j : 256-j] is the
        # [128, 128] matrix with ones in column j, zeros elsewhere.
        Z = consts.tile([P, 2 * P], bf16)
        nc.gpsimd.memset(Z[:], 0.0)
        nc.gpsimd.memset(Z[:, P : P + 1], 1.0)
        ident = consts.tile([P, P], f32)
        make_identity(nc, ident[:])
        ones_row = consts.tile([1, P], f32)
        nc.gpsimd.memset(ones_row[:], 1.0)
        one_one = consts.tile([1, 1], f32)
        nc.gpsimd.memset(one_one[:], 1.0)

        for b in [b for _ in range(repeat) for b in range(BPC)]:
            # ---- load x[b]: channels onto partitions in 4 chunks ----
            xTf = xpool.tile([P, NCC, NP], f32)
            nc.sync.dma_start(
                out=xTf[:], in_=x_d[b].rearrange("(a p) n -> p a n", p=P)
            )
            xT = xpool.tile([P, NCC, NP], bf16)
            nc.vector.tensor_copy(xT[:], xTf[:])
            # fp32 tensor holding the exact bf16-rounded values (per-part
            # scalar operands must be fp32; matching values keep D exact
            # in bf16 arithmetic)
            xTb = xpool.tile([P, NCC, NP], f32)
            nc.vector.tensor_copy(xTb[:], xT[:])

            # ---- S[p] = sum_c x[c, p] via PE ones-column reduction ----
            W0 = Z[:, P : 2 * P]
            ps_a = psum.tile([P, 512], f32, tag="ps_a")
            ps_b = psum.tile([P, NP - 512], f32, tag="ps_b")
            for cc in range(NCC):
                nc.tensor.matmul(
                    ps_a[:], W0, xT[:, cc, 0:512],
                    start=(cc == 0), stop=(cc == NCC - 1),
                )
                nc.tensor.matmul(
                    ps_b[:], W0, xT[:, cc, 512:NP],
                    start=(cc == 0), stop=(cc == NCC - 1),
                )
            S_row = spool.tile([1, NP], f32)
            nc.scalar.copy(S_row[0:1, 0:512], ps_a[0:1, :])
            nc.scalar.copy(S_row[0:1, 512:NP], ps_b[0:1, :])
            # replicate S across partitions:  ones_col^T @ S_row
            pr_a = psum.tile([P, 512], f32, tag="ps_a")
            pr_b = psum.tile([P, NP - 512], f32, tag="ps_b")
            nc.tensor.matmul(pr_a[:], ones_row[:], S_row[0:1, 0:512])
            nc.tensor.matmul(pr_b[:], ones_row[:], S_row[0:1, 512:NP])
            Srep = spool.tile([P, NP], f32)
            nc.vector.tensor_copy(Srep[:, 0:512], pr_a[:])
            nc.vector.tensor_copy(Srep[:, 512:NP], pr_b[:])
            # S as per-partition columns: S_col[:, g] = S[g*128 + part]
            S_col = spool.tile([P, NPB], f32)
            nc.gpsimd.memset(S_col[:], 0.0)
            for g in range(NPB):
                pc = tpsum.tile([P, 1], f32, tag="pt")
                nc.tensor.matmul(
                    pc[: rcnt(g), :],
                    S_row[0:1, g * P : g * P + rcnt(g)],
                    one_one[:],
                )
                nc.scalar.copy(S_col[: rcnt(g), g : g + 1], pc[: rcnt(g), :])

            # ---- strictly-upper MinSum triangle ----
            D_sb = dpool.tile([P, NPB, NP2], f32)
            nc.gpsimd.memset(D_sb[:, :, NP:NP2], 0.0)
            for g in range(NPB):
                qn = rcnt(g)
                if g * P < 512:
                    ps_a = psum.tile([P, 512], f32, tag="ps_a")
                else:
                    ps_a = None
                ps_b = psum.tile([P, NP - 512], f32, tag="ps_b")
                for jq in range(qn):
                    q = g * P + jq
                    qs = q + (q & 1)  # 4-byte-aligned even start
                    if qs >= NP:
                        continue
                    Wj = Z[:, P - jq : 2 * P - jq]
                    st = jq == 0
                    use_act = (q % ACT_MOD) >= ACT_CUT
                    for cc in range(NCC):
                        A = apool.tile([P, NP], bf16)
                        if use_act:
                            # min(x_p, x_q) = x_q - relu(x_q - x_p)
                            t = apool.tile([P, NP], bf16, tag="t")
                            nc.scalar.activation(
                                t[:, qs:NP], xT[:, cc, qs:NP], Relu,
                                bias=xTb[:, cc, q : q + 1], scale=-1.0,
                            )
                            nc.scalar.activation(
                                A[:, qs:NP], t[:, qs:NP], Ident,
                                bias=xTb[:, cc, q : q + 1], scale=-1.0,
                            )
                        else:
                            nc.vector.tensor_scalar(
                                A[:, qs:NP],
                                xT[:, cc, qs:NP],
                                xTb[:, cc, q : q + 1],
                                None,
                                AluOpType.min,
                            )
                        if qs < 512:
                            nc.tensor.matmul(
                                ps_a[:, qs:512], Wj, A[:, qs:512],
                                start=(st and cc == 0),
                                stop=False,
                            )
                        nc.tensor.matmul(
                            ps_b[:, max(qs, 512) - 512 : NP - 512],
                            Wj,
                            A[:, max(qs, 512) : NP],
                            start=(st and cc == 0),
                            stop=False,
                        )
                # close the accumulation group on every element with a
                # full-range zero-weight matmul (adds 0, marks stop)
                Wz = Z[:, 0:P]
                lo = g * P
                if lo < 512:
                    nc.tensor.matmul(
                        ps_a[:, lo:512], Wz, xT[:, 0, lo:512],
                        start=False, stop=True,
                    )
                nc.tensor.matmul(
                    ps_b[:, max(lo, 512) - 512 : NP - 512],
                    Wz,
                    xT[:, 0, max(lo, 512) : NP],
                    start=False, stop=True,
                )
                if lo < 512:
                    nc.scalar.copy(D_sb[:, g, lo:512], ps_a[:, lo:512])
                nc.scalar.copy(
                    D_sb[:, g, max(lo, 512) : NP],
                    ps_b[:, max(lo, 512) - 512 : NP - 512],
                )

            # ---- mirror: M = U + U^T (strict upper -> symmetric) ----
            for i in range(NPB):
                # clean the diag block: keep strictly-upper, zero the rest
                db = D_sb[:, i, blk(i)]
                nc.gpsimd.affine_select(
                    out=db, in_=db,
                    compare_op=AluOpType.is_gt,
                    fill=0.0, base=0,
                    pattern=[[1, P]],
                    channel_multiplier=-1,
                )
                pt = tpsum.tile([P, P], f32, tag="pt")
                nc.tensor.transpose(pt[:], db, ident[:])
                nc.vector.scalar_tensor_tensor(
                    db, db, 0.0, pt[:], AluOpType.add, AluOpType.add
                )
                for j in range(i + 1, NPB):
                    ub = D_sb[:, i, blk(j)]
                    pt = tpsum.tile([P, P], f32, tag="pt")
                    nc.tensor.transpose(pt[:], ub, ident[:])
                    nc.scalar.copy(D_sb[:, j, blk(i)], pt[:])

            # ---- D_partial = -2*M + S_p  (rows of S replicated) ----
            for g in range(NPB):
                nc.vector.scalar_tensor_tensor(
                    D_sb[:, g, 0:NP], D_sb[:, g, 0:NP], -2.0, Srep[:],
                    AluOpType.mult, AluOpType.add,
                )

            # ---- row sums: s = sum_p D_partial + 782 * S_q ----
            s_raw = spool.tile([P, NPB], f32)
            for g in range(NPB):
                nc.vector.reduce_sum(s_raw[:, g : g + 1], D_sb[:, g, 0:NP], X)
            s_t = spool.tile([P, NPB], f32)
            nc.vector.scalar_tensor_tensor(
                s_t[:], S_col[:], float(NP - 2), s_raw[:],
                AluOpType.mult, AluOpType.add,
            )
            # the reference clamps with max(sum, 1e-12); also guards the
            # 112 dead rows of the tail block
            nc.vector.tensor_scalar_max(s_t[:], s_t[:], 1e-12)
            r_t = spool.tile([P, NPB], f32)
            nc.vector.reciprocal(r_t[:], s_t[:])

            # ---- scale: Dt[q, p] = (D_partial + S_q) / s_q ----
            for g in range(NPB):
                nc.vector.tensor_scalar(
                    D_sb[:, g, 0:NP], D_sb[:, g, 0:NP],
                    S_col[:, g : g + 1], r_t[:, g : g + 1],
                    AluOpType.add, AluOpType.mult,
                )
                # exact-zero diagonal (true D has zero diagonal)
                db = D_sb[:, g, blk(g)]
                nc.gpsimd.affine_select(
                    out=db, in_=db,
                    compare_op=AluOpType.not_equal,
                    fill=0.0, base=0,
                    pattern=[[1, P]],
                    channel_multiplier=-1,
                )

            # ---- z iteration: z0 = s/NP, z' = Dt_stored^T @ z ----
            z = zpool.tile([P, NPB], f32)
            nc.gpsimd.memset(z[:], 0.0)
            nc.vector.tensor_scalar_mul(z[:, 0:6], s_t[:, 0:6], 1.0 / NP)
            nc.vector.tensor_scalar_mul(
                z[:TAIL, 6:7], s_t[:TAIL, 6:7], 1.0 / NP
            )
            for _ in range(N_ITER):
                zp = zpsum.tile([P, NPB], f32)
                zn = zpool.tile([P, NPB], f32)
                nc.gpsimd.memset(zn[:], 0.0)
                for i in range(NPB):
                    m = rcnt(i)
                    for j in range(NPB):
                        nc.tensor.matmul(
                            zp[:m, i : i + 1],
                            D_sb[:, j, i * P : i * P + m],
                            z[:, j : j + 1],
                            start=(j == 0),
                            stop=(j == NPB - 1),
                        )
                    nc.scalar.copy(zn[:m, i : i + 1], zp[:m, i : i + 1])
                z = zn

            # ---- M = z_10 / s ----
            v = zpool.tile([P, NPB], f32)
            nc.vector.tensor_tensor(v[:], z[:], r_t[:], AluOpType.mult)
            for j in range(6):
                nc.sync.dma_start(
                    out=out_d[b, j * P : (j + 1) * P], in_=v[:, j]
                )
            nc.sync.dma_start(out=out_d[b, 6 * P : NP], in_=v[:TAIL, 6])

    nc.compile()
    return nc


def _get_program(repeat: int = 1):
    key = ("nc", repeat)
    if key not in _CACHE:
        _CACHE[key] = _build_program(repeat)
    return _CACHE[key]


def kernel(x: np.ndarray) -> np.ndarray:
    from concourse.bass_utils import run_bass_kernel_spmd

    assert x.shape == (B, C, H, W), x.shape
    nc = _get_program()
    xs = np.ascontiguousarray(x.reshape(B, C, NP), dtype=np.float32)
    in_maps = [
        {"x": xs[i * BPC : (i + 1) * BPC]} for i in range(N_CORES)
    ]
    res = run_bass_kernel_spmd(nc, in_maps, list(range(N_CORES)))
    out = np.concatenate([r["out"] for r in res.results], axis=0)
    return out.reshape(B, H, W).astype(x.dtype, copy=False)


if __name__ == "__main__":
    rng = np.random.default_rng(0)
    xt = rng.standard_normal((B, C, H, W), dtype=np.float32)
    print(kernel(xt).shape)

